# revision 13
# baseline (speedup 1.0000x reference)
"""Cross-image contrastive loss on 8 TRN2 NeuronCores.

Strategy (row-parallel over N=4096 pixels, 512 rows/core, rows sorted by label):
  - All matmuls run in fp8 DoubleRow perf mode (0.5 cyc/col): the d=64
    contraction is split into two 32-feature k-tiles riding the PE's fp8
    double-pumping, with a 33rd contraction row carrying a per-(row,segment)
    additive bias that implements the S2 label mask inside the matmul
    (masked-out entries reach exp() at -686 and underflow to exactly 0).
  - exp work is split across three engines per 1024-col unit:
    Scalar (activation Exp with fused accum row-sum), and DVE (Schraudolph
    int32 exponential via tensor_scalar) whose row-sum pass is offloaded to
    GPSIMD (tensor_reduce over the bitcast f32 tile).
  - S2 (label-matched exp sums vs Fjj) needs one matmul + one scalar
    activation per 128-row block: rows and columns are label-sorted, each
    block spans <=2 labels, its two 256-col segments are masked by the
    33rd-row bias; zero-padded columns contribute exp(0)=1 and are
    subtracted exactly on the host.
  - Device ships the [128, 20] per-block partial-sum table; the host (O(N))
    computes Z, log Z, the per-pixel weights and the final reduction.
"""

import math
import sys

import numpy as np

sys.path.insert(0, "/opt/trn_rl_repo")

import ml_dtypes

TAU = 0.07
EPS = 1e-4
L = 19
D = 64
N = 4096
NCORES = 8
P = N // NCORES  # 512 rows per core
PB = P // 128  # 4 partition blocks per core
U = 4  # S1 units (1024 cols each) per block
SW = 256  # S2 segment width (max label count in data ~238)
K = 34  # contraction: 2x32 feature k-tiles + 1 bias row + 1 zero pad row
# (even partition count: fp8 DoubleRow double-pumps partition pairs)
FSCL = 8.0  # fp8 feature scale; psum = FSCL^2 * logits
MSC = FSCL * FSCL
ESC = 1.0 / (MSC * TAU)  # exp scale applied to psum
BIAS_RAW = -6.0  # pre-FSCL bias-row weight; -6*8 * 8*ESC = -85.7 pre-exp
# small enough that Schraudolph's int argument stays positive (so masked
# entries land at ~1e-33 instead of wrapping), big enough to vanish vs Z
# Schraudolph: exp(u) ~ bitcast_f32(int32(A*psum + B)); C calibrated mean-zero
SCH_C = 0.0562
SCH_A = (1 << 23) * ESC / math.log(2.0)
SCH_B = (1 << 23) * (127.0 - SCH_C) + 0.5

# per-(block, unit) consumer: 'a' scalar activation + fused accum,
# 'v' DVE Schraudolph tensor_scalar + fold-with-accum tensor_tensor_reduce
ASSIGN = {
    (0, 0): "v", (0, 1): "a", (0, 2): "v", (0, 3): "a",
    (1, 0): "v", (1, 1): "a", (1, 2): "a", (1, 3): "a",
    (2, 0): "v", (2, 1): "a", (2, 2): "v", (2, 3): "a",
    (3, 0): "v", (3, 1): "a", (3, 2): "a", (3, 3): "a",
}
NC_OUT = PB * (U + 1)  # acc cols: per block, U S1 cols + 1 S2 col

_compiled = None
_LDW_PATCHED = False


def _enable_ldw_opt():
    """No-op: walrus's LDW dedup rejects DoubleRow InstLdweights
    ("InstLdweights is not compatible with LDW optimization")."""
    return
    global _LDW_PATCHED
    if _LDW_PATCHED:
        return
    from concourse import bass_utils

    orig = bass_utils.run_command

    def patched(cmd, *a, **kw):
        if isinstance(cmd, list):
            cmd = [
                "--enable-ldw-opt=true" if c == "--enable-ldw-opt=false" else c
                for c in cmd
            ]
        return orig(cmd, *a, **kw)

    bass_utils.run_command = patched
    _LDW_PATCHED = True


def _build():
    from concourse import bacc, mybir, tile

    f32 = mybir.dt.float32
    i32 = mybir.dt.int32
    bf16 = mybir.dt.bfloat16
    f8 = mybir.dt.float8e4
    Exp = mybir.ActivationFunctionType.Exp
    X = mybir.AxisListType.X
    add = mybir.AluOpType.add
    mult = mybir.AluOpType.mult
    DR = mybir.MatmulPerfMode.DoubleRow

    nc = bacc.Bacc("TRN2", target_bir_lowering=False, debug=False)

    lhsP_d = nc.dram_tensor("lhsP", (K, PB * 256), f8, kind="ExternalInput")
    rhs1_d = nc.dram_tensor("rhs1", (K, 2 * U * 1024), f8, kind="ExternalInput")
    rhs2_d = nc.dram_tensor("rhs2", (K, PB * 1024), f8, kind="ExternalInput")
    out_d = nc.dram_tensor("out", (128, NC_OUT), f32, kind="ExternalOutput")

    with tile.TileContext(nc) as tc:
        with (
            tc.tile_pool(name="res", bufs=1) as res,
            tc.tile_pool(name="scr", bufs=2) as scr,
            tc.tile_pool(name="yi", bufs=3) as yip,
            tc.tile_pool(name="ps1", bufs=3, space="PSUM") as ps1,
            tc.tile_pool(name="ps2", bufs=2, space="PSUM") as ps2,
        ):
            lhsP_sb = res.tile([K, PB * 256], f8, tag="lhsP")
            rhs1_sb = res.tile([K, 2 * U * 1024], f8, tag="rhs1")
            rhs2_sb = res.tile([K, PB * 1024], f8, tag="rhs2")
            acc = res.tile([128, NC_OUT], f32, tag="acc")
            zeros = res.tile([128, 1], f32, tag="zeros")

            # input DMAs spread across the two HWDGE sequencers so the
            # DIRECT2D descriptor setups overlap; critical pieces first
            nc.scalar.dma_start(lhsP_sb[:], lhsP_d[:])
            nc.sync.dma_start(rhs1_sb[:, 0:2048], rhs1_d[:, 0:2048])
            nc.sync.dma_start(rhs2_sb[:], rhs2_d[:])
            nc.sync.dma_start(rhs1_sb[:, 2048:8192], rhs1_d[:, 2048:8192])
            nc.vector.memset(zeros[:], 0.0)

            def wt(t):
                return lhsP_sb[:, t * 256 : (t + 1) * 256].rearrange(
                    "p (two m) -> p two m", two=2
                )

            def s1_unit(t, u, kind):
                ps = ps1.tile([128, 1024], f32, tag="mm")
                for h in range(2):
                    c0 = (2 * u + h) * 1024
                    rhs = rhs1_sb[:, c0 : c0 + 1024].rearrange(
                        "p (two n) -> p two n", two=2
                    )
                    nc.tensor.matmul(
                        ps[:, h * 512 : (h + 1) * 512],
                        wt(t),
                        rhs,
                        start=True,
                        stop=True,
                        perf_mode=DR,
                    )
                col = t * (U + 1) + u
                if kind == "a":
                    dump = scr.tile([128, 1024], bf16, tag="dump")
                    nc.scalar.activation(
                        dump[:],
                        ps[:],
                        Exp,
                        bias=zeros[:],
                        scale=ESC,
                        accum_out=acc[:, col : col + 1],
                    )
                else:
                    yi = yip.tile([128, 1024], i32, tag="yi")
                    nc.vector.tensor_scalar(yi[:], ps[:], SCH_A, SCH_B, mult, add)
                    nc.vector.tensor_reduce(
                        acc[:, col : col + 1], yi[:].bitcast(f32), axis=X, op=add
                    )

            def s2_unit(t):
                ps = ps2.tile([128, 512], f32, tag="mm2")
                rhs = rhs2_sb[:, t * 1024 : (t + 1) * 1024].rearrange(
                    "p (two n) -> p two n", two=2
                )
                nc.tensor.matmul(
                    ps[:], wt(t), rhs, start=True, stop=True, perf_mode=DR
                )
                col = t * (U + 1) + U
                dump = scr.tile([128, 512], bf16, tag="dump2")
                nc.scalar.activation(
                    dump[:],
                    ps[:],
                    Exp,
                    bias=zeros[:],
                    scale=ESC,
                    accum_out=acc[:, col : col + 1],
                )

            for t in range(PB):
                s1_unit(t, 0, ASSIGN[(t, 0)])
                s2_unit(t)
                for u in range(1, U):
                    s1_unit(t, u, ASSIGN[(t, u)])

            nc.sync.dma_start(out_d[:], acc[:])

    nc.compile()
    return nc


def _make_in_maps(features_i, features_ii, features_jj, i, ii, jj):
    f8 = ml_dtypes.float8_e4m3fn
    Fi = features_i.reshape(D, N).astype(np.float32)
    Fii = features_ii.reshape(D, N).astype(np.float32)
    Fjj = features_jj.reshape(D, N).astype(np.float32)
    lab = i.reshape(-1)
    ii_f = ii.reshape(-1)
    jj_f = jj.reshape(-1)

    cnt_ii = np.bincount(ii_f, minlength=L).astype(np.float32)
    cnt_jj = np.bincount(jj_f, minlength=L).astype(np.float32)
    wl = cnt_ii / (cnt_ii + cnt_jj + EPS)  # [L]

    perm_r = np.argsort(lab, kind="stable")
    lab_s = lab[perm_r]
    Fi_s = Fi[:, perm_r]
    perm_c = np.argsort(jj_f, kind="stable")
    jj_s = jj_f[perm_c]
    Fjj_s = Fjj[:, perm_c]
    jstart = np.searchsorted(jj_s, np.arange(L), "left")
    jend = np.searchsorted(jj_s, np.arange(L), "right")

    dsum = (Fi * (Fii + Fjj)).sum(0) / TAU  # [N] diag1+diag2
    dsum_s = dsum[perm_r]
    w_s = wl[lab_s]

    # rhs1: shared across cores; chunk j holds [ktile0(512) | ktile1(512)]
    rhs1 = np.zeros((K, 2 * N), np.float32)
    for j in range(N // 512):
        cs = slice(j * 512, (j + 1) * 512)
        rhs1[0:32, j * 1024 : j * 1024 + 512] = Fii[0:32, cs]
        rhs1[32:33, j * 1024 : j * 1024 + 512] = 0.0
        rhs1[0:32, j * 1024 + 512 : (j + 1) * 1024] = Fii[32:64, cs]
    rhs1_f8 = (rhs1 * FSCL).astype(f8)

    in_maps = []
    host = []  # per-core (w_rows, dsum_rows, zoff_rows) per block
    for c in range(NCORES):
        lhsP = np.zeros((K, PB * 256), np.float32)
        rhs2 = np.zeros((K, PB * 1024), np.float32)
        zoff = np.zeros((PB, 128), np.float64)
        for t in range(PB):
            rows = slice((PB * c + t) * 128, (PB * c + t + 1) * 128)
            base = t * 256
            lhsP[0:32, base : base + 128] = Fi_s[0:32, rows]
            lhsP[0:32, base + 128 : base + 256] = Fi_s[32:64, rows]
            blk_lab = lab_s[rows]
            dl = np.unique(blk_lab)
            assert len(dl) <= 2, f"block {PB * c + t} spans {len(dl)} labels"
            b2 = t * 1024
            for s in range(2):
                seg = slice(b2 + s * SW, b2 + (s + 1) * SW)
                seg_k1 = slice(b2 + 512 + s * SW, b2 + 512 + (s + 1) * SW)
                if s < len(dl):
                    l = int(dl[s])
                    n_l = jend[l] - jstart[l]
                    assert n_l <= SW, f"label {l} has {n_l} cols > SW={SW}"
                    rhs2[0:32, b2 + s * SW : b2 + s * SW + n_l] = Fjj_s[
                        0:32, jstart[l] : jend[l]
                    ]
                    rhs2[0:32, b2 + 512 + s * SW : b2 + 512 + s * SW + n_l] = (
                        Fjj_s[32:64, jstart[l] : jend[l]]
                    )
                    in_seg = blk_lab == l
                else:
                    in_seg = np.zeros(128, bool)
                # segment marker rides k-tile s's 33rd contraction row
                rhs2[32, (seg if s == 0 else seg_k1)] = 1.0
                # bias-row weights: 0 for in-segment rows, big-negative else
                lhsP[32, base + s * 128 : base + s * 128 + 128] = np.where(
                    in_seg, 0.0, BIAS_RAW
                )
            zoff[t] = -(SW - cnt_jj[blk_lab])
        host.append(
            (
                w_s[PB * c * 128 : PB * (c + 1) * 128].astype(np.float64),
                dsum_s[PB * c * 128 : PB * (c + 1) * 128].astype(np.float64),
                zoff,
            )
        )
        in_maps.append(
            {
                "lhsP": (lhsP * FSCL).astype(f8),
                "rhs1": rhs1_f8,
                "rhs2": (rhs2 * FSCL).astype(f8),
            }
        )
    return in_maps, host


def _finish(out, host_c):
    """Host epilogue for one core: out [128, NC_OUT] -> loss partial."""
    w, dsum, zoff = host_c
    acc = out.astype(np.float64).reshape(128, PB, U + 1)
    part = 0.0
    for t in range(PB):
        Z = acc[:, t, :].sum(axis=1) + zoff[t] + EPS
        rows = slice(t * 128, (t + 1) * 128)
        part += (w[rows] * (2.0 * np.log(Z) - dsum[rows])).sum()
    return part / N


def kernel(features_i, features_ii, features_jj, i, ii, jj):
    global _compiled
    _enable_ldw_opt()
    from concourse import bass_utils

    if _compiled is None:
        _compiled = _build()
    in_maps, host = _make_in_maps(
        features_i, features_ii, features_jj, i, ii, jj
    )
    results = bass_utils.run_bass_kernel_spmd(
        _compiled, in_maps, core_ids=list(range(NCORES))
    )
    total = 0.0
    for c, r in enumerate(results.results):
        total += _finish(np.asarray(r["out"]), host[c])
    return np.array(total, dtype=np.float32)


# revision 14
# speedup vs baseline: 1.1298x; 1.1298x over previous
"""Cross-image contrastive loss on 8 TRN2 NeuronCores.

Strategy (row-parallel over N=4096 pixels, 512 rows/core, rows sorted by label):
  - fp8 matmuls with block-diagonal K=128 weights: the two 64-row halves of
    each 128-row block ride one instruction (top-left 64x64 = features of
    rows 0..63, bottom-right = rows 64..127, rhs carries Fii twice), so the
    full PE array is engaged and all tensors are 128-partition (all 16 DMA
    queues spread the input load).
  - exp work is split per 1024-col unit between the Scalar engine
    (activation Exp with fused accum row-sum) and the DVE (Schraudolph int32
    exponential via tensor_scalar); for DVE units, GPSIMD folds the two yi
    halves (tensor_tensor add on the bitcast f32 view) so the DVE reduce
    only sees 512 elements.
  - S2 (label-matched exp sums vs Fjj): rows and columns label-sorted, each
    128-row block spans <=2 labels -> one 512-col matmul per block (2
    segments of 256), scalar exp to a bf16 dump, DVE applies the per-row 0/1
    segment mask as a fused multiply+accumulate; zero-padded columns
    contribute exp(0)=1 and are subtracted exactly on the host.
  - Device ships the [128, 24] per-block partial-sum table; the host (O(N))
    computes Z, log Z, the per-pixel weights and the final reduction.
"""

import math
import sys

import numpy as np

sys.path.insert(0, "/opt/trn_rl_repo")

import ml_dtypes

TAU = 0.07
EPS = 1e-4
L = 19
D = 64
N = 4096
NCORES = 8
P = N // NCORES  # 512 rows per core
PB = P // 128  # 4 partition blocks per core
U = 4  # S1 units (1024 cols each) per block
SW = 256  # S2 segment width (max label count in data ~238)
FSCL = 8.0  # fp8 feature scale; psum = FSCL^2 * logits
MSC = FSCL * FSCL
ESC = 1.0 / (MSC * TAU)  # exp scale applied to psum
# Schraudolph: exp(u) ~ bitcast_f32(int32(A*psum + B)); C calibrated mean-zero
SCH_C = 0.0562
SCH_A = (1 << 23) * ESC / math.log(2.0)
SCH_B = (1 << 23) * (127.0 - SCH_C) + 0.5

# per-(block, unit) consumer: 'a' scalar activation + fused accum,
# 'v' DVE Schraudolph + GPSIMD half-fold + DVE reduce
ASSIGN = {
    (0, 0): "v", (0, 1): "a", (0, 2): "v", (0, 3): "a",
    (1, 0): "v", (1, 1): "a", (1, 2): "a", (1, 3): "a",
    (2, 0): "v", (2, 1): "a", (2, 2): "v", (2, 3): "a",
    (3, 0): "v", (3, 1): "a", (3, 2): "a", (3, 3): "a",
}
GP_FOLD = True  # fold yi halves on GPSIMD before the DVE reduce
NC_OUT = PB * (U + 2)  # acc cols: per block, U S1 cols + 2 S2 cols

_compiled = None


def _build():
    from concourse import bacc, mybir, tile

    f32 = mybir.dt.float32
    i32 = mybir.dt.int32
    bf16 = mybir.dt.bfloat16
    f8 = mybir.dt.float8e4
    Exp = mybir.ActivationFunctionType.Exp
    X = mybir.AxisListType.X
    add = mybir.AluOpType.add
    mult = mybir.AluOpType.mult

    nc = bacc.Bacc("TRN2", target_bir_lowering=False, debug=False)

    lhsP_d = nc.dram_tensor("lhsP", (128, PB * 128), f8, kind="ExternalInput")
    rhs1_d = nc.dram_tensor("rhs1", (128, N), f8, kind="ExternalInput")
    rhs2_d = nc.dram_tensor("rhs2", (128, PB * 512), f8, kind="ExternalInput")
    small_d = nc.dram_tensor("small", (128, 9), f32, kind="ExternalInput")
    out_d = nc.dram_tensor("out", (128, NC_OUT), f32, kind="ExternalOutput")

    with tile.TileContext(nc) as tc:
        with (
            tc.tile_pool(name="res", bufs=1) as res,
            tc.tile_pool(name="scr", bufs=2) as scr,
            tc.tile_pool(name="yi", bufs=3) as yip,
            tc.tile_pool(name="ps1", bufs=3, space="PSUM") as ps1,
            tc.tile_pool(name="ps2", bufs=2, space="PSUM") as ps2,
        ):
            lhsP_sb = res.tile([128, PB * 128], f8, tag="lhsP")
            rhs1_sb = res.tile([128, N], f8, tag="rhs1")
            rhs2_sb = res.tile([128, PB * 512], f8, tag="rhs2")
            small_sb = res.tile([128, 9], f32, tag="small")
            acc = res.tile([128, NC_OUT], f32, tag="acc")

            # input DMAs split across the two HWDGE sequencers; critical first
            nc.scalar.dma_start(lhsP_sb[:], lhsP_d[:])
            nc.sync.dma_start(rhs1_sb[:, 0:1024], rhs1_d[:, 0:1024])
            nc.scalar.dma_start(rhs2_sb[:], rhs2_d[:])
            nc.sync.dma_start(rhs1_sb[:, 1024:4096], rhs1_d[:, 1024:4096])
            nc.scalar.dma_start(small_sb[:], small_d[:])

            mask = small_sb[:, 0:8]  # col t*2+s: 1.0 where row in segment
            zeros = small_sb[:, 8:9]

            def wt(t):
                return lhsP_sb[:, t * 128 : (t + 1) * 128]

            def s1_unit(t, u, kind):
                ps = ps1.tile([128, 1024], f32, tag="mm")
                for h in range(2):
                    c0 = (2 * u + h) * 512
                    nc.tensor.matmul(
                        ps[:, h * 512 : (h + 1) * 512],
                        wt(t),
                        rhs1_sb[:, c0 : c0 + 512],
                        start=True,
                        stop=True,
                    )
                col = t * (U + 2) + u
                if kind == "a":
                    dump = scr.tile([128, 1024], bf16, tag="dump")
                    nc.scalar.activation(
                        dump[:],
                        ps[:],
                        Exp,
                        bias=zeros,
                        scale=ESC,
                        accum_out=acc[:, col : col + 1],
                    )
                else:
                    yi = yip.tile([128, 1024], i32, tag="yi")
                    nc.vector.tensor_scalar(yi[:], ps[:], SCH_A, SCH_B, mult, add)
                    if GP_FOLD:
                        gf = scr.tile([128, 512], f32, tag="gf")
                        nc.gpsimd.tensor_tensor(
                            gf[:],
                            yi[:, 0:512].bitcast(f32),
                            yi[:, 512:1024].bitcast(f32),
                            add,
                        )
                        nc.vector.tensor_reduce(
                            acc[:, col : col + 1], gf[:], axis=X, op=add
                        )
                    else:
                        nc.vector.tensor_reduce(
                            acc[:, col : col + 1], yi[:].bitcast(f32), axis=X, op=add
                        )

            def s2_unit(t):
                ps = ps2.tile([128, 512], f32, tag="mm2")
                nc.tensor.matmul(
                    ps[:],
                    wt(t),
                    rhs2_sb[:, t * 512 : (t + 1) * 512],
                    start=True,
                    stop=True,
                )
                dump = scr.tile([128, 512], bf16, tag="dump2")
                nc.scalar.activation(dump[:], ps[:], Exp, bias=zeros, scale=ESC)
                for s in range(2):
                    col = t * (U + 2) + U + s
                    tmp = scr.tile([128, SW], bf16, tag="tmp2")
                    nc.vector.tensor_scalar(
                        tmp[:],
                        dump[:, s * SW : (s + 1) * SW],
                        mask[:, t * 2 + s : t * 2 + s + 1],
                        None,
                        mult,
                        add,
                        accum_out=acc[:, col : col + 1],
                    )

            for t in range(PB):
                s1_unit(t, 0, ASSIGN[(t, 0)])
                s2_unit(t)
                for u in range(1, U):
                    s1_unit(t, u, ASSIGN[(t, u)])

            nc.sync.dma_start(out_d[:], acc[:])

    nc.compile()
    return nc


def _make_in_maps(features_i, features_ii, features_jj, i, ii, jj):
    f8 = ml_dtypes.float8_e4m3fn
    Fi = features_i.reshape(D, N).astype(np.float32)
    Fii = features_ii.reshape(D, N).astype(np.float32)
    Fjj = features_jj.reshape(D, N).astype(np.float32)
    lab = i.reshape(-1)
    ii_f = ii.reshape(-1)
    jj_f = jj.reshape(-1)

    cnt_ii = np.bincount(ii_f, minlength=L).astype(np.float32)
    cnt_jj = np.bincount(jj_f, minlength=L).astype(np.float32)
    wl = cnt_ii / (cnt_ii + cnt_jj + EPS)  # [L]

    perm_r = np.argsort(lab, kind="stable")
    lab_s = lab[perm_r]
    Fi_s = Fi[:, perm_r]
    perm_c = np.argsort(jj_f, kind="stable")
    jj_s = jj_f[perm_c]
    Fjj_s = Fjj[:, perm_c]
    jstart = np.searchsorted(jj_s, np.arange(L), "left")
    jend = np.searchsorted(jj_s, np.arange(L), "right")

    dsum = (Fi * (Fii + Fjj)).sum(0) / TAU  # [N] diag1+diag2
    dsum_s = dsum[perm_r]
    w_s = wl[lab_s]

    rhs1 = np.zeros((128, N), np.float32)
    rhs1[0:D] = Fii
    rhs1[D : 2 * D] = Fii  # duplicate for the bottom-right weight half
    rhs1_f8 = (rhs1 * FSCL).astype(f8)

    in_maps = []
    host = []  # per-core (w_rows, dsum_rows, zoff) with zoff [PB, 128]
    for c in range(NCORES):
        lhsP = np.zeros((128, PB * 128), np.float32)
        rhs2 = np.zeros((128, PB * 512), np.float32)
        small = np.zeros((128, 9), np.float32)
        zoff = np.zeros((PB, 128), np.float64)
        for t in range(PB):
            rows = slice((PB * c + t) * 128, (PB * c + t + 1) * 128)
            base = t * 128
            blk = Fi_s[:, rows]  # [64, 128]
            lhsP[0:64, base : base + 64] = blk[:, 0:64]
            lhsP[64:128, base + 64 : base + 128] = blk[:, 64:128]
            blk_lab = lab_s[rows]
            dl = np.unique(blk_lab)
            assert len(dl) <= 2, f"block {PB * c + t} spans {len(dl)} labels"
            for s in range(2):
                if s < len(dl):
                    l = int(dl[s])
                    n_l = jend[l] - jstart[l]
                    assert n_l <= SW, f"label {l} has {n_l} cols > SW={SW}"
                    seg = Fjj_s[:, jstart[l] : jend[l]]
                    rhs2[0:64, t * 512 + s * SW : t * 512 + s * SW + n_l] = seg
                    rhs2[64:128, t * 512 + s * SW : t * 512 + s * SW + n_l] = seg
                    small[:, t * 2 + s] = (blk_lab == l).astype(np.float32)
            zoff[t] = -(SW - cnt_jj[blk_lab])
        host.append(
            (
                w_s[PB * c * 128 : PB * (c + 1) * 128].astype(np.float64),
                dsum_s[PB * c * 128 : PB * (c + 1) * 128].astype(np.float64),
                zoff,
            )
        )
        in_maps.append(
            {
                "lhsP": (lhsP * FSCL).astype(f8),
                "rhs1": rhs1_f8,
                "rhs2": (rhs2 * FSCL).astype(f8),
                "small": small,
            }
        )
    return in_maps, host


def _finish(out, host_c):
    """Host epilogue for one core: out [128, NC_OUT] -> loss partial."""
    w, dsum, zoff = host_c
    acc = out.astype(np.float64).reshape(128, PB, U + 2)
    part = 0.0
    for t in range(PB):
        Z = acc[:, t, :].sum(axis=1) + zoff[t] + EPS
        rows = slice(t * 128, (t + 1) * 128)
        part += (w[rows] * (2.0 * np.log(Z) - dsum[rows])).sum()
    return part / N


def kernel(features_i, features_ii, features_jj, i, ii, jj):
    global _compiled
    from concourse import bass_utils

    if _compiled is None:
        _compiled = _build()
    in_maps, host = _make_in_maps(
        features_i, features_ii, features_jj, i, ii, jj
    )
    results = bass_utils.run_bass_kernel_spmd(
        _compiled, in_maps, core_ids=list(range(NCORES))
    )
    total = 0.0
    for c, r in enumerate(results.results):
        total += _finish(np.asarray(r["out"]), host[c])
    return np.array(total, dtype=np.float32)


# revision 17
# speedup vs baseline: 1.1625x; 1.0290x over previous
"""Cross-image contrastive loss on 8 TRN2 NeuronCores.

Strategy (row-parallel over N=4096 pixels, 512 rows/core, rows sorted by label):
  - fp8 matmuls with block-diagonal K=128 weights: the two 64-row halves of
    each 128-row block ride one instruction (top-left 64x64 = features of
    rows 0..63, bottom-right = rows 64..127, rhs carries Fii twice), so the
    full PE array is engaged and all tensors are 128-partition (all 16 DMA
    queues spread the input load).
  - exp work is split per 1024-col unit between the Scalar engine
    (activation Exp with fused accum row-sum) and the DVE (Schraudolph int32
    exponential via tensor_scalar); for DVE units, GPSIMD folds the two yi
    halves (tensor_tensor add on the bitcast f32 view) so the DVE reduce
    only sees 512 elements.
  - S2 (label-matched exp sums vs Fjj): rows and columns label-sorted, each
    128-row block spans <=2 labels -> one 512-col matmul per block (2
    segments of 256), scalar exp to a bf16 dump, DVE applies the per-row 0/1
    segment mask as a fused multiply+accumulate; zero-padded columns
    contribute exp(0)=1 and are subtracted exactly on the host.
  - Device ships the [128, 24] per-block partial-sum table; the host (O(N))
    computes Z, log Z, the per-pixel weights and the final reduction.
"""

import math
import sys

import numpy as np

sys.path.insert(0, "/opt/trn_rl_repo")

import ml_dtypes

TAU = 0.07
EPS = 1e-4
L = 19
D = 64
N = 4096
NCORES = 8
P = N // NCORES  # 512 rows per core
PB = P // 128  # 4 partition blocks per core
U = 4  # S1 units (1024 cols each) per block
SW = 256  # S2 segment width (max label count in data ~238)
FSCL = 8.0  # fp8 feature scale; psum = FSCL^2 * logits
MSC = FSCL * FSCL
ESC = 1.0 / (MSC * TAU)  # exp scale applied to psum
# Schraudolph: exp(u) ~ bitcast_f32(int32(A*psum + B)); C calibrated mean-zero
SCH_C = 0.0562
SCH_A = (1 << 23) * ESC / math.log(2.0)
SCH_B = (1 << 23) * (127.0 - SCH_C) + 0.5

# per-(block, unit) consumer: 'a' scalar activation + fused accum,
# 'v' DVE Schraudolph + 2-level GPSIMD fold + DVE reduce
ASSIGN = {
    (0, 0): "v", (0, 1): "a", (0, 2): "v", (0, 3): "a",
    (1, 0): "v", (1, 1): "a", (1, 2): "v", (1, 3): "a",
    (2, 0): "v", (2, 1): "a", (2, 2): "v", (2, 3): "a",
    (3, 0): "v", (3, 1): "a", (3, 2): "a", (3, 3): "a",
}
S2_DVE = {3}  # blocks whose S2 path runs Schraudolph on DVE (no scalar)
N_WARM = 5  # consumerless PE warm-up matmuls to start the p-state ramp
NC_OUT = PB * (U + 2)  # acc cols: per block, U S1 cols + 2 S2 cols

# Schraudolph output for a zero argument (bf16 rounding happens later in the
# masked multiply whose output is bf16; the accumulate runs in fp32)
_SCH0 = float(
    np.array(int(np.float32(SCH_B)), dtype=np.int32).view(np.float32)
)

_compiled = None


def _build():
    from concourse import bacc, mybir, tile

    f32 = mybir.dt.float32
    i32 = mybir.dt.int32
    bf16 = mybir.dt.bfloat16
    f8 = mybir.dt.float8e4
    Exp = mybir.ActivationFunctionType.Exp
    X = mybir.AxisListType.X
    add = mybir.AluOpType.add
    mult = mybir.AluOpType.mult

    nc = bacc.Bacc("TRN2", target_bir_lowering=False, debug=False)
    InstLoadActFuncSet = mybir.InstLoadActFuncSet

    lhsP_d = nc.dram_tensor("lhsP", (128, PB * 128), f8, kind="ExternalInput")
    rhs1_d = nc.dram_tensor("rhs1", (128, N), f8, kind="ExternalInput")
    rhs2_d = nc.dram_tensor("rhs2", (128, PB * 512), f8, kind="ExternalInput")
    small_d = nc.dram_tensor("small", (128, 9), f32, kind="ExternalInput")
    out_d = nc.dram_tensor("out", (128, NC_OUT), f32, kind="ExternalOutput")

    with tile.TileContext(nc) as tc:
        with (
            tc.tile_pool(name="res", bufs=1) as res,
            tc.tile_pool(name="scr", bufs=2) as scr,
            tc.tile_pool(name="yi", bufs=3) as yip,
            tc.tile_pool(name="ps1", bufs=3, space="PSUM") as ps1,
            tc.tile_pool(name="ps2", bufs=1, space="PSUM") as ps2,
            tc.tile_pool(name="psw", bufs=1, space="PSUM") as psw,
        ):
            lhsP_sb = res.tile([128, PB * 128], f8, tag="lhsP")
            rhs1_sb = res.tile([128, N], f8, tag="rhs1")
            rhs2_sb = res.tile([128, PB * 512], f8, tag="rhs2")
            small_sb = res.tile([128, 9], f32, tag="small")
            acc = res.tile([128, NC_OUT], f32, tag="acc")

            warm_in = res.tile([128, 512], f8, tag="warm_in")

            # input DMAs split across the two HWDGE sequencers; critical first
            nc.scalar.dma_start(lhsP_sb[:], lhsP_d[:])
            nc.sync.dma_start(rhs1_sb[:, 0:1024], rhs1_d[:, 0:1024])
            # manual act-table load here keeps the auto-insert pass from
            # hoisting a load above scalar's DMA descriptor setups
            nc.scalar.add_instruction(
                InstLoadActFuncSet(
                    name=nc.get_next_instruction_name(),
                    act_func_set_id=0,  # exp_and_others
                    ins=[],
                    outs=[],
                )
            )
            nc.scalar.dma_start(rhs2_sb[:], rhs2_d[:])
            nc.sync.dma_start(rhs1_sb[:, 1024:4096], rhs1_d[:, 1024:4096])
            nc.scalar.dma_start(small_sb[:], small_d[:])

            # consumerless warm-up matmuls keep the PE streaming through the
            # input-DMA window so the 2.4GHz p-state ramp starts immediately
            nc.vector.memset(warm_in[:], 0.25)
            wps = psw.tile([128, 512], f32, tag="warm")
            for _ in range(N_WARM):
                nc.tensor.matmul(
                    wps[:], warm_in[:, 0:128], warm_in[:], start=True, stop=True
                )

            mask = small_sb[:, 0:8]  # col t*2+s: 1.0 where row in segment
            zeros = small_sb[:, 8:9]

            def wt(t):
                return lhsP_sb[:, t * 128 : (t + 1) * 128]

            def s1_unit(t, u, kind):
                ps = ps1.tile([128, 1024], f32, tag="mm")
                for h in range(2):
                    c0 = (2 * u + h) * 512
                    nc.tensor.matmul(
                        ps[:, h * 512 : (h + 1) * 512],
                        wt(t),
                        rhs1_sb[:, c0 : c0 + 512],
                        start=True,
                        stop=True,
                    )
                col = t * (U + 2) + u
                if kind == "a":
                    dump = scr.tile([128, 1024], bf16, tag="dump")
                    nc.scalar.activation(
                        dump[:],
                        ps[:],
                        Exp,
                        bias=zeros,
                        scale=ESC,
                        accum_out=acc[:, col : col + 1],
                    )
                else:
                    yi = yip.tile([128, 1024], i32, tag="yi")
                    nc.vector.tensor_scalar(yi[:], ps[:], SCH_A, SCH_B, mult, add)
                    gf = scr.tile([128, 512], f32, tag="gf")
                    nc.gpsimd.tensor_tensor(
                        gf[:],
                        yi[:, 0:512].bitcast(f32),
                        yi[:, 512:1024].bitcast(f32),
                        add,
                    )
                    gf2 = scr.tile([128, 256], f32, tag="gf2")
                    nc.gpsimd.tensor_tensor(
                        gf2[:], gf[:, 0:256], gf[:, 256:512], add
                    )
                    nc.vector.tensor_reduce(
                        acc[:, col : col + 1], gf2[:], axis=X, op=add
                    )

            def s2_unit(t):
                ps = ps2.tile([128, 512], f32, tag="mm2")
                nc.tensor.matmul(
                    ps[:],
                    wt(t),
                    rhs2_sb[:, t * 512 : (t + 1) * 512],
                    start=True,
                    stop=True,
                )
                if t in S2_DVE:
                    yi2 = yip.tile([128, 512], i32, tag="yi2")
                    nc.vector.tensor_scalar(
                        yi2[:], ps[:], SCH_A, SCH_B, mult, add
                    )
                    src_t = yi2[:].bitcast(f32)
                else:
                    dump = scr.tile([128, 512], bf16, tag="dump2")
                    nc.scalar.activation(
                        dump[:], ps[:], Exp, bias=zeros, scale=ESC
                    )
                    src_t = dump[:]
                for s in range(2):
                    col = t * (U + 2) + U + s
                    tmp = scr.tile([128, SW], bf16, tag="tmp2")
                    nc.vector.tensor_scalar(
                        tmp[:],
                        src_t[:, s * SW : (s + 1) * SW],
                        mask[:, t * 2 + s : t * 2 + s + 1],
                        None,
                        mult,
                        add,
                        accum_out=acc[:, col : col + 1],
                    )

            for t in range(PB):
                s1_unit(t, 0, ASSIGN[(t, 0)])
                s2_unit(t)
                for u in range(1, U):
                    s1_unit(t, u, ASSIGN[(t, u)])

            nc.sync.dma_start(out_d[:], acc[:])

    nc.compile()
    return nc


def _make_in_maps(features_i, features_ii, features_jj, i, ii, jj):
    f8 = ml_dtypes.float8_e4m3fn
    Fi = features_i.reshape(D, N).astype(np.float32)
    Fii = features_ii.reshape(D, N).astype(np.float32)
    Fjj = features_jj.reshape(D, N).astype(np.float32)
    lab = i.reshape(-1)
    ii_f = ii.reshape(-1)
    jj_f = jj.reshape(-1)

    cnt_ii = np.bincount(ii_f, minlength=L).astype(np.float32)
    cnt_jj = np.bincount(jj_f, minlength=L).astype(np.float32)
    wl = cnt_ii / (cnt_ii + cnt_jj + EPS)  # [L]

    perm_r = np.argsort(lab, kind="stable")
    lab_s = lab[perm_r]
    Fi_s = Fi[:, perm_r]
    perm_c = np.argsort(jj_f, kind="stable")
    jj_s = jj_f[perm_c]
    Fjj_s = Fjj[:, perm_c]
    jstart = np.searchsorted(jj_s, np.arange(L), "left")
    jend = np.searchsorted(jj_s, np.arange(L), "right")

    dsum = (Fi * (Fii + Fjj)).sum(0) / TAU  # [N] diag1+diag2
    dsum_s = dsum[perm_r]
    w_s = wl[lab_s]

    rhs1 = np.zeros((128, N), np.float32)
    rhs1[0:D] = Fii
    rhs1[D : 2 * D] = Fii  # duplicate for the bottom-right weight half
    rhs1_f8 = (rhs1 * FSCL).astype(f8)

    in_maps = []
    host = []  # per-core (w_rows, dsum_rows, zoff) with zoff [PB, 128]
    for c in range(NCORES):
        lhsP = np.zeros((128, PB * 128), np.float32)
        rhs2 = np.zeros((128, PB * 512), np.float32)
        small = np.zeros((128, 9), np.float32)
        zoff = np.zeros((PB, 128), np.float64)
        for t in range(PB):
            rows = slice((PB * c + t) * 128, (PB * c + t + 1) * 128)
            base = t * 128
            blk = Fi_s[:, rows]  # [64, 128]
            lhsP[0:64, base : base + 64] = blk[:, 0:64]
            lhsP[64:128, base + 64 : base + 128] = blk[:, 64:128]
            blk_lab = lab_s[rows]
            dl = np.unique(blk_lab)
            assert len(dl) <= 2, f"block {PB * c + t} spans {len(dl)} labels"
            for s in range(2):
                if s < len(dl):
                    l = int(dl[s])
                    n_l = jend[l] - jstart[l]
                    assert n_l <= SW, f"label {l} has {n_l} cols > SW={SW}"
                    seg = Fjj_s[:, jstart[l] : jend[l]]
                    rhs2[0:64, t * 512 + s * SW : t * 512 + s * SW + n_l] = seg
                    rhs2[64:128, t * 512 + s * SW : t * 512 + s * SW + n_l] = seg
                    small[:, t * 2 + s] = (blk_lab == l).astype(np.float32)
            # padded columns contribute exp(0): 1.0 from the scalar engine,
            # the Schraudolph-at-zero value on the DVE path
            pad1 = _SCH0 if t in S2_DVE else 1.0
            zoff[t] = -(SW - cnt_jj[blk_lab]) * pad1
        host.append(
            (
                w_s[PB * c * 128 : PB * (c + 1) * 128].astype(np.float64),
                dsum_s[PB * c * 128 : PB * (c + 1) * 128].astype(np.float64),
                zoff,
            )
        )
        in_maps.append(
            {
                "lhsP": (lhsP * FSCL).astype(f8),
                "rhs1": rhs1_f8,
                "rhs2": (rhs2 * FSCL).astype(f8),
                "small": small,
            }
        )
    return in_maps, host


def _finish(out, host_c):
    """Host epilogue for one core: out [128, NC_OUT] -> loss partial."""
    w, dsum, zoff = host_c
    acc = out.astype(np.float64).reshape(128, PB, U + 2)
    part = 0.0
    for t in range(PB):
        Z = acc[:, t, :].sum(axis=1) + zoff[t] + EPS
        rows = slice(t * 128, (t + 1) * 128)
        part += (w[rows] * (2.0 * np.log(Z) - dsum[rows])).sum()
    return part / N


def kernel(features_i, features_ii, features_jj, i, ii, jj):
    global _compiled
    from concourse import bass_utils

    if _compiled is None:
        _compiled = _build()
    in_maps, host = _make_in_maps(
        features_i, features_ii, features_jj, i, ii, jj
    )
    results = bass_utils.run_bass_kernel_spmd(
        _compiled, in_maps, core_ids=list(range(NCORES))
    )
    total = 0.0
    for c, r in enumerate(results.results):
        total += _finish(np.asarray(r["out"]), host[c])
    return np.array(total, dtype=np.float32)
